# revision 9
# baseline (speedup 1.0000x reference)
"""Trainium2 Bass kernel for nn_CONV_COMPLEX_v1 (complex CNN + FC heads).

Self-contained: builds all Toeplitz/block-diag weights in numpy, compiles one
SPMD Bass/Tile program, shards batch over 8 cores, gathers [4096, 3] output.

See inline layout comments; validated against the reference via layouts.py
(numpy mirror) to ~1e-7.
"""
import sys
if '/opt/trn_rl_repo' not in sys.path:
    sys.path.insert(0, '/opt/trn_rl_repo')

import numpy as np
import concourse.bass as bass
import concourse.bacc as bacc
import concourse.mybir as mybir
from concourse import tile, dve_ops
from concourse.dve_spec import Spec, Src0, Src1, C0, C1, select
from concourse.dve_ops import RECIPROCAL_APPROX_FAST
from concourse.bass_utils import run_bass_kernel_spmd

AF = mybir.ActivationFunctionType
OP = mybir.AluOpType
dt = mybir.dt
F32 = dt.float32

SLOPE = 0.05
B, T, C = 4096, 64, 14
NCORES = 8
BC = B // NCORES          # 512 batch per core
R = BC * 14               # 7168 rows per core
CH = 512                  # row chunk
NCHUNK = R // CH          # 14
BIG = 1e30

# ---------------------------------------------------------------- custom op
def _register(op):
    if op.name in dve_ops._SUB_OPCODE_FOR_NAME:
        return
    dve_ops.OPS.append(op)
    dve_ops._SUB_OPCODE_FOR_NAME[op.name] = max(dve_ops._SUB_OPCODE_FOR_NAME.values()) + 1
    dve_ops.CUSTOM_DVE_SPECS[op.name] = op.spec


CLEAKY_SELMUL = dve_ops.DveOp(
    "CLEAKY_SELMUL",
    Spec(body=select(Src1 < C0, Src0 * Src1, C1),
         reference=lambda in0, in1, s0, s1, imm2: np.where(
             in1 < s0, (in0 * in1).astype(np.float32), np.float32(s1)).astype(np.float32)),
    subdim=False,
    uops_sha={"v3": "7defef3488ac79a9", "v4": "1911aa2b7a375206"},
)
_register(CLEAKY_SELMUL)

# ---------------------------------------------------------------- weights (numpy)
GROUPS = [
    (0, 3), (1, 4), (2, 5), (None, 6),
    (0, 7), (1, 8), (2, 9), (None, 10),
    ((0, 1), 13), ((1, 2), 13), ((2, 0), 13),
    ((0, 1), 11), ((1, 2), 12), ((2, 0), 13),
]


def _xn_row(comp, t):
    if t < 32:
        return comp * 32 + t
    return 64 + comp * 32 + (t - 32)


def _build_weights(inp):
    w = {}
    # conv32 (branch1): W32 [129, 64]: cols 0:32 re-out (co*3+p, pad 24:32), 32:64 im-out
    wr, wi = inp['conv32_w'].real, inp['conv32_w'].imag
    br, bi = inp['conv32_b'].real, inp['conv32_b'].imag
    W32 = np.zeros((129, 64), np.float32)
    for co in range(8):
        for p in range(3):
            m = co * 3 + p
            for k in range(32):
                t = 16 * p + k
                W32[_xn_row(0, t), m] += wr[co, 0, k]
                W32[_xn_row(1, t), m] += -wi[co, 0, k]
                W32[_xn_row(0, t), 32 + m] += wi[co, 0, k]
                W32[_xn_row(1, t), 32 + m] += wr[co, 0, k]
            W32[128, m] = br[co]
            W32[128, 32 + m] = bi[co]
    W32[128, 56:64] = 1.0
    w['W32'] = W32

    # conv1: W1 [65, 1024], tile q cols q*128..; q = parity*4 + comp_o*2 + cihalf
    wr, wi = inp['c3w1'].real, inp['c3w1'].imag
    br, bi = inp['c3b1'].real, inp['c3b1'].imag
    W1 = np.zeros((65, 1024), np.float32)
    for parity in range(2):
        for comp_o in range(2):
            for cihalf in range(2):
                q = parity * 4 + comp_o * 2 + cihalf
                for ci_loc in range(8):
                    ci = cihalf * 8 + ci_loc
                    for l2 in range(16):
                        l = 2 * l2 + parity
                        m = q * 128 + ci_loc * 16 + l2
                        for k in range(3):
                            tp = (l + k - 1) % 32
                            if comp_o == 0:
                                W1[tp, m] += wr[ci, 0, k]
                                W1[32 + tp, m] += -wi[ci, 0, k]
                            else:
                                W1[tp, m] += wi[ci, 0, k]
                                W1[32 + tp, m] += wr[ci, 0, k]
                        W1[64, m] = br[ci] if comp_o == 0 else bi[ci]
    w['W1'] = W1

    # conv2: W2 [4, 128, 1024] kc-major; W2B [1, 1024]
    wr, wi = inp['c3w2'].real, inp['c3w2'].imag
    br, bi = inp['c3b2'].real, inp['c3b2'].imag
    W2 = np.zeros((4, 128, 1024), np.float32)
    W2B = np.zeros((1, 1024), np.float32)
    for comp_o in range(2):
        for parity in range(2):
            for l3half in range(2):
                mt = comp_o * 4 + parity * 2 + l3half
                for l3_loc in range(4):
                    l = 2 * (l3half * 4 + l3_loc) + parity
                    for co in range(32):
                        m = mt * 128 + l3_loc * 32 + co
                        for ci in range(16):
                            for k in range(3):
                                li = (l + k - 1) % 16
                                krow = (ci % 8) * 16 + li
                                if comp_o == 0:
                                    W2[0 + ci // 8, krow, m] += wr[co, ci, k]
                                    W2[2 + ci // 8, krow, m] += -wi[co, ci, k]
                                else:
                                    W2[0 + ci // 8, krow, m] += wi[co, ci, k]
                                    W2[2 + ci // 8, krow, m] += wr[co, ci, k]
                        W2B[0, m] = br[co] if comp_o == 0 else bi[co]
    w['W2'] = W2
    w['W2B'] = W2B

    # conv3: W3 [4, 128, 512]; W3B [1, 512]; M tiles: mt = comp_o*2 + lhalf, part = l_loc*32+co
    wr, wi = inp['c3w3'].real, inp['c3w3'].imag
    br, bi = inp['c3b3'].real, inp['c3b3'].imag
    W3 = np.zeros((4, 128, 512), np.float32)
    W3B = np.zeros((1, 512), np.float32)
    for comp_o in range(2):
        for lhalf in range(2):
            mt = comp_o * 2 + lhalf
            for l_loc in range(4):
                l = lhalf * 4 + l_loc
                for co in range(32):
                    m = mt * 128 + l_loc * 32 + co
                    for ci in range(32):
                        for k in range(3):
                            j = (l + k - 1) % 8
                            kc = 0 if j < 4 else 1
                            krow = (j % 4) * 32 + ci
                            if comp_o == 0:
                                W3[0 + kc, krow, m] += wr[co, ci, k]
                                W3[2 + kc, krow, m] += -wi[co, ci, k]
                            else:
                                W3[0 + kc, krow, m] += wi[co, ci, k]
                                W3[2 + kc, krow, m] += wr[co, ci, k]
                    W3B[0, m] = br[co] if comp_o == 0 else bi[co]
    w['W3'] = W3
    w['W3B'] = W3B

    # FC1: [15, 128, 256]; K idx = ch*64 + comp*32 + f (blocks of 7 tiles each), tile14 = bias
    wr, wi = inp['hw1'].real, inp['hw1'].imag
    br, bi = inp['hb1'].real, inp['hb1'].imag
    FC1 = np.zeros((15, 128, 256), np.float32)
    for h in range(3):
        for o in range(40):
            m = h * 40 + o
            for f in range(56):
                for ch in range(14):
                    i = f * 14 + ch
                    if f < 24:
                        base, fl = 0, f
                    else:
                        base, fl = 7, f - 24
                    kr = ch * 64 + 0 * 32 + fl
                    ki = ch * 64 + 1 * 32 + fl
                    FC1[base + kr // 128, kr % 128, m] += wr[h, o, i]
                    FC1[base + ki // 128, ki % 128, m] += -wi[h, o, i]
                    FC1[base + kr // 128, kr % 128, 128 + m] += wi[h, o, i]
                    FC1[base + ki // 128, ki % 128, 128 + m] += wr[h, o, i]
            FC1[14, 0, m] = br[h, o]
            FC1[14, 0, 128 + m] = bi[h, o]
    FC1[14, 0, 248:256] = 1.0
    w['FC1'] = FC1

    # FC2: [2, 128, 1024] (k-tile0 from z1re, 1 from z1im); FC2B [1, 1024]
    wr, wi = inp['hw2'].real, inp['hw2'].imag
    br, bi = inp['hb2'].real, inp['hb2'].imag
    FC2 = np.zeros((2, 128, 1024), np.float32)
    FC2B = np.zeros((1, 1024), np.float32)
    for h in range(3):
        for o in range(160):
            m = h * 160 + o
            for i in range(40):
                k = h * 40 + i
                FC2[0, k, m] += wr[h, o, i]
                FC2[1, k, m] += -wi[h, o, i]
                FC2[0, k, 512 + m] += wi[h, o, i]
                FC2[1, k, 512 + m] += wr[h, o, i]
            FC2B[0, m] = br[h, o]
            FC2B[0, 512 + m] = bi[h, o]
    FC2B[0, 992:1024] = 1.0
    w['FC2'] = FC2
    w['FC2B'] = FC2B

    # FC3: [8, 128, 256] (tiles 0..3 = z2re rows, 4..7 = z2im); FC3B [1, 256]
    wr, wi = inp['hw3'].real, inp['hw3'].imag
    br, bi = inp['hb3'].real, inp['hb3'].imag
    FC3 = np.zeros((8, 128, 256), np.float32)
    FC3B = np.zeros((1, 256), np.float32)
    for h in range(3):
        for o in range(40):
            m = h * 40 + o
            for i in range(160):
                k = h * 160 + i
                FC3[k // 128, k % 128, m] += wr[h, o, i]
                FC3[4 + k // 128, k % 128, m] += -wi[h, o, i]
                FC3[k // 128, k % 128, 128 + m] += wi[h, o, i]
                FC3[4 + k // 128, k % 128, 128 + m] += wr[h, o, i]
            FC3B[0, m] = br[h, o]
            FC3B[0, 128 + m] = bi[h, o]
    FC3B[0, 248:256] = 1.0
    w['FC3'] = FC3
    w['FC3B'] = FC3B

    # FC4: [2, 128, 128] (tile0 z3re, tile1 z3im); FC4B [1, 128]; M: re 0:64 (60+4pad), im 64:128
    wr, wi = inp['hw4'].real, inp['hw4'].imag
    br, bi = inp['hb4'].real, inp['hb4'].imag
    FC4 = np.zeros((2, 128, 128), np.float32)
    FC4B = np.zeros((1, 128), np.float32)
    for h in range(3):
        for o in range(20):
            m = h * 20 + o
            for i in range(40):
                k = h * 40 + i
                FC4[0, k, m] += wr[h, o, i]
                FC4[1, k, m] += -wi[h, o, i]
                FC4[0, k, 64 + m] += wi[h, o, i]
                FC4[1, k, 64 + m] += wr[h, o, i]
            FC4B[0, m] = br[h, o]
            FC4B[0, 64 + m] = bi[h, o]
    FC4B[0, 124:128] = 1.0
    w['FC4'] = FC4
    w['FC4B'] = FC4B

    # FC5: [128, 3] (rows 0:64 z4re, 64:128 z4im); FC5B [1, 3]
    wr, wi = inp['hw5'].real, inp['hw5'].imag
    br = inp['hb5'].real
    FC5 = np.zeros((128, 3), np.float32)
    FC5B = np.zeros((1, 3), np.float32)
    for h in range(3):
        for i in range(20):
            k = h * 20 + i
            FC5[k, h] += wr[h, 0, i]
            FC5[64 + k, h] += -wi[h, 0, i]
        FC5B[0, h] = br[h, 0]
    w['FC5'] = FC5
    w['FC5B'] = FC5B
    return w


# ---------------------------------------------------------------- bass program
_CACHE = {}


def _build_program():
    nc = bacc.Bacc("TRN2", target_bir_lowering=False, debug=False, num_devices=NCORES)

    x_d = nc.dram_tensor("x", [BC, T, C], F32, kind="ExternalInput").ap()
    wd = {}
    for name, shape in [
        ('W32', [129, 64]), ('W1', [65, 1024]),
        ('W2', [4, 128, 1024]), ('W2B', [1, 1024]),
        ('W3', [4, 128, 512]), ('W3B', [1, 512]),
        ('FC1', [15, 128, 256]), ('FC2', [2, 128, 1024]), ('FC2B', [1, 1024]),
        ('FC3', [8, 128, 256]), ('FC3B', [1, 256]),
        ('FC4', [2, 128, 128]), ('FC4B', [1, 128]),
        ('FC5', [128, 3]), ('FC5B', [1, 3]),
    ]:
        wd[name] = nc.dram_tensor(name, shape, F32, kind="ExternalInput").ap()
    out_d = nc.dram_tensor("out", [BC, 3], F32, kind="ExternalOutput").ap()

    with tile.TileContext(nc) as tc:
        _emit(nc, tc, x_d, wd, out_d)

    nc.compile()
    return nc


def _emit(nc, tc, x_d, wd, out_d):
    import contextlib

    # ---------------- global pools
    with contextlib.ExitStack() as gctx:
        cpool = gctx.enter_context(tc.tile_pool(name="consts", bufs=1))
        fb = gctx.enter_context(tc.tile_pool(name="fb", bufs=1))
        ps = gctx.enter_context(tc.tile_pool(name="ps", bufs=1, space="PSUM"))

        ones = cpool.tile([1, CH], F32, tag="ones", name="ones")
        nc.gpsimd.memset(ones[:], 1.0)
        bias_s = cpool.tile([128, 1], F32, tag="bias_s", name="bias_s")
        bias_c = cpool.tile([128, 1], F32, tag="bias_c", name="bias_c")
        nc.gpsimd.memset(bias_s[:], 0.475 * np.pi)
        nc.gpsimd.memset(bias_c[:], 0.025 * np.pi)

        # Fball [128, R]: 0:24 b1re, 32:56 b1im, 64:96 b2re, 96:128 b2im
        fball = fb.tile([128, R], F32, tag="fball", name="fball")

        def cleaky(pool, re_ap, im_ap, P, n, out_re=None, out_im=None, bufs=2):
            rcp = pool.tile([P, n], F32, tag="ck_rcp", name="ck_rcp", bufs=bufs)
            nc.vector.reciprocal_approx_fast(out=rcp[:], in_=im_ap)
            g = pool.tile([P, n], F32, tag="ck_g", name="ck_g", bufs=bufs)
            nc.vector._custom_dve(CLEAKY_SELMUL, out=g[:], in0=re_ap, in1=rcp[:],
                                  s0=0.0, s1=-BIG)
            b = pool.tile([P, n], F32, tag="ck_b", name="ck_b", bufs=bufs)
            nc.scalar.activation(b[:], g[:], AF.Arctan)
            s = pool.tile([P, n], F32, tag="ck_s", name="ck_s", bufs=bufs)
            c = pool.tile([P, n], F32, tag="ck_c", name="ck_c", bufs=bufs)
            nc.scalar.activation(s[:], b[:], AF.Sin, scale=0.95, bias=bias_s[0:P, :])
            nc.scalar.activation(c[:], b[:], AF.Sin, scale=-0.95, bias=bias_c[0:P, :])
            p1 = pool.tile([P, n], F32, tag="ck_p1", name="ck_p1", bufs=bufs)
            q1 = pool.tile([P, n], F32, tag="ck_q1", name="ck_q1", bufs=bufs)
            if out_re is None:
                ore_t = pool.tile([P, n], F32, tag="ck_ore", name="ck_ore", bufs=bufs)
                out_re = ore_t[:]
            else:
                ore_t = None
            if out_im is None:
                oim_t = pool.tile([P, n], F32, tag="ck_oim", name="ck_oim", bufs=bufs)
                out_im = oim_t[:]
            else:
                oim_t = None
            nc.vector.tensor_tensor(out=p1[:], in0=re_ap, in1=c[:], op=OP.mult)
            nc.vector.tensor_tensor(out=q1[:], in0=im_ap, in1=s[:], op=OP.mult)
            nc.vector.tensor_tensor(out=out_re, in0=p1[:], in1=q1[:], op=OP.subtract)
            p2 = pool.tile([P, n], F32, tag="ck_p1", name="ck_p1", bufs=bufs)
            q2 = pool.tile([P, n], F32, tag="ck_q1", name="ck_q1", bufs=bufs)
            nc.vector.tensor_tensor(out=p2[:], in0=re_ap, in1=s[:], op=OP.mult)
            nc.vector.tensor_tensor(out=q2[:], in0=im_ap, in1=c[:], op=OP.add if False else OP.mult)
            nc.vector.tensor_tensor(out=out_im, in0=p2[:], in1=q2[:], op=OP.add)
            return out_re, out_im

        # ================= conv phase =================
        with contextlib.ExitStack() as cctx:
            big = cctx.enter_context(tc.tile_pool(name="big", bufs=1))

            # XN build (two 64-partition tiles: lo = t 0..31, hi = t 32..63)
            xnlo = big.tile([64, R], F32, tag="xnlo", name="xnlo")
            xnhi = big.tile([64, R], F32, tag="xnhi", name="xnhi")
            with tc.tile_pool(name="x0pool", bufs=1) as x0pool:
                x0 = x0pool.tile([64, R], F32, tag="x0", name="x0")
                nc.sync.dma_start(x0.rearrange("t (b c) -> t b c", c=14),
                                  x_d.rearrange("b t c -> t b c"))

                def xplane(c, lo):
                    return x0[lo:lo + 32].rearrange("t (b c) -> t b c", c=14)[:, :, c]

                for ch, (respec, imc) in enumerate(GROUPS):
                    for lo, xt_ in ((0, xnlo), (32, xnhi)):
                        dst = xt_.rearrange("p (b c) -> p b c", c=14)
                        if respec is None:
                            nc.vector.memset(dst[0:32, :, ch], 0.0)
                        elif isinstance(respec, tuple):
                            nc.any.tensor_tensor(out=dst[0:32, :, ch],
                                                 in0=xplane(respec[0], lo),
                                                 in1=xplane(respec[1], lo), op=OP.subtract)
                        else:
                            nc.any.tensor_copy(dst[0:32, :, ch], xplane(respec, lo))
                        nc.any.tensor_copy(dst[32:64, :, ch], xplane(imc, lo))

            wpool = cctx.enter_context(tc.tile_pool(name="cw", bufs=1))
            work = cctx.enter_context(tc.tile_pool(name="cwork", bufs=2))
            x2p = cctx.enter_context(tc.tile_pool(name="x2p", bufs=1))

            # conv-phase weights
            wt = {}
            for name in ('W2', 'W2B', 'W3', 'W3B'):
                ap = wd[name]
                shape = ap.shape
                if len(shape) == 2:
                    t = wpool.tile(list(shape), F32, tag=f"w_{name}", name=f"w_{name}")
                    nc.sync.dma_start(t[:], ap[:])
                    wt[name] = t
                else:
                    tiles = []
                    for i in range(shape[0]):
                        t = wpool.tile(list(shape[1:]), F32, tag=f"w_{name}_{i}",
                                       name=f"w_{name}_{i}")
                        nc.sync.dma_start(t[:], ap[i])
                        tiles.append(t)
                    wt[name] = tiles
            w32lo = wpool.tile([64, 64], F32, tag="w32lo", name="w32lo")
            w32hi = wpool.tile([64, 64], F32, tag="w32hi", name="w32hi")
            w32b = wpool.tile([1, 64], F32, tag="w32b", name="w32b")
            nc.sync.dma_start(w32lo[:], wd['W32'][0:64, :])
            nc.sync.dma_start(w32hi[:], wd['W32'][64:128, :])
            nc.sync.dma_start(w32b[:], wd['W32'][128:129, :])
            w1a = wpool.tile([64, 1024], F32, tag="w1a", name="w1a")
            w1b = wpool.tile([1, 1024], F32, tag="w1b", name="w1b")
            nc.sync.dma_start(w1a[:], wd['W1'][0:64, :])
            nc.sync.dma_start(w1b[:], wd['W1'][64:65, :])

            for cidx in range(NCHUNK):
                cs = slice(cidx * CH, (cidx + 1) * CH)
                rhs_lo = xnlo[:, cs]
                rhs_hi = xnhi[:, cs]

                # --- branch1 conv32
                p_re = ps.tile([32, CH], F32, tag="cps", name="cps", bufs=6)
                p_im = ps.tile([32, CH], F32, tag="cps", name="cps", bufs=6)
                nc.tensor.matmul(p_re[:], w32lo[:, 0:32], rhs_lo, start=True, stop=False)
                nc.tensor.matmul(p_re[:], w32hi[:, 0:32], rhs_hi, start=False, stop=False)
                nc.tensor.matmul(p_re[:], w32b[:, 0:32], ones[:], start=False, stop=True)
                nc.tensor.matmul(p_im[:], w32lo[:, 32:64], rhs_lo, start=True, stop=False)
                nc.tensor.matmul(p_im[:], w32hi[:, 32:64], rhs_hi, start=False, stop=False)
                nc.tensor.matmul(p_im[:], w32b[:, 32:64], ones[:], start=False, stop=True)
                b1ore, b1oim = cleaky(work, p_re[:], p_im[:], 32, CH)
                nc.any.tensor_copy(fball[0:24, cs], b1ore[0:24, :])
                nc.any.tensor_copy(fball[32:56, cs], b1oim[0:24, :])

                # --- conv1: emit in cleaky-pair order (0,2),(1,3),(4,6),(5,7)
                c1 = [None] * 8
                X2 = [None] * 4
                for t in range(4):
                    X2[t] = x2p.tile([128, CH], F32, tag=f"x2_{t}", name=f"x2_{t}")

                def c1mm(q):
                    pt = ps.tile([128, CH], F32, tag="cps", name="cps", bufs=6)
                    nc.tensor.matmul(pt[:], w1a[:, q * 128:(q + 1) * 128], rhs_hi,
                                     start=True, stop=False)
                    nc.tensor.matmul(pt[:], w1b[:, q * 128:(q + 1) * 128], ones[:],
                                     start=False, stop=True)
                    c1[q] = pt

                ev = {}
                od = {}
                for cihalf in range(2):
                    c1mm(0 + cihalf); c1mm(2 + cihalf)
                    ev[cihalf] = cleaky(work, c1[cihalf][:], c1[2 + cihalf][:], 128, CH)
                    c1mm(4 + cihalf); c1mm(6 + cihalf)
                    od[cihalf] = cleaky(work, c1[4 + cihalf][:], c1[6 + cihalf][:], 128, CH)
                    nc.vector.tensor_tensor(out=X2[cihalf][:], in0=ev[cihalf][0],
                                            in1=od[cihalf][0], op=OP.max)
                    nc.vector.tensor_tensor(out=X2[2 + cihalf][:], in0=ev[cihalf][1],
                                            in1=od[cihalf][1], op=OP.max)

                # --- conv2: pairs (0,4),(2,6),(1,5),(3,7)
                c2 = [None] * 8
                X3 = [None] * 4
                for t in range(4):
                    X3[t] = x2p.tile([128, CH], F32, tag=f"x3_{t}", name=f"x3_{t}")

                def c2mm(mt):
                    pt = ps.tile([128, CH], F32, tag="cps", name="cps", bufs=6)
                    for kc in range(4):
                        nc.tensor.matmul(pt[:], wt['W2'][kc][:, mt * 128:(mt + 1) * 128],
                                         X2[kc][:], start=(kc == 0), stop=False)
                    nc.tensor.matmul(pt[:], wt['W2B'][:, mt * 128:(mt + 1) * 128],
                                     ones[:], start=False, stop=True)
                    c2[mt] = pt

                pooled2 = {}
                for l3half in range(2):
                    c2mm(0 + l3half); c2mm(4 + l3half)
                    e_ = cleaky(work, c2[0 + l3half][:], c2[4 + l3half][:], 128, CH)
                    c2mm(2 + l3half); c2mm(6 + l3half)
                    o_ = cleaky(work, c2[2 + l3half][:], c2[6 + l3half][:], 128, CH)
                    nc.vector.tensor_tensor(out=X3[l3half][:], in0=e_[0], in1=o_[0], op=OP.max)
                    nc.vector.tensor_tensor(out=X3[2 + l3half][:], in0=e_[1], in1=o_[1], op=OP.max)

                # --- conv3: pairs (0,2),(1,3)
                c3 = [None] * 4

                def c3mm(mt):
                    pt = ps.tile([128, CH], F32, tag="cps", name="cps", bufs=6)
                    for kc in range(4):
                        nc.tensor.matmul(pt[:], wt['W3'][kc][:, mt * 128:(mt + 1) * 128],
                                         X3[kc][:], start=(kc == 0), stop=False)
                    nc.tensor.matmul(pt[:], wt['W3B'][:, mt * 128:(mt + 1) * 128],
                                     ones[:], start=False, stop=True)
                    c3[mt] = pt

                c3mm(0); c3mm(2)
                R0c, I0c = cleaky(work, c3[0][:], c3[2][:], 128, CH)
                c3mm(1); c3mm(3)
                R1c, I1c = cleaky(work, c3[1][:], c3[3][:], 128, CH)
                for src0, src1, fbase in ((R0c, R1c, 64), (I0c, I1c, 96)):
                    G = work.tile([128, CH], F32, tag="c3g", name="c3g", bufs=1)
                    nc.vector.tensor_tensor(out=G[:], in0=src0, in1=src1, op=OP.max)
                    t64 = work.tile([64, CH], F32, tag="c3t64", name="c3t64", bufs=1)
                    nc.any.tensor_copy(t64[:], G[64:128, :])
                    H = work.tile([64, CH], F32, tag="c3h", name="c3h", bufs=1)
                    nc.vector.tensor_tensor(out=H[:], in0=G[0:64, :], in1=t64[:], op=OP.max)
                    t32 = work.tile([32, CH], F32, tag="c3t32", name="c3t32", bufs=1)
                    nc.any.tensor_copy(t32[:], H[32:64, :])
                    Fx = work.tile([32, CH], F32, tag="c3f", name="c3f", bufs=1)
                    nc.vector.tensor_tensor(out=Fx[:], in0=H[0:32, :], in1=t32[:], op=OP.max)
                    nc.any.tensor_copy(fball[fbase:fbase + 32, cs], Fx[:])

        # ================= FC phase =================
        with contextlib.ExitStack() as fctx:
            fwp = fctx.enter_context(tc.tile_pool(name="fw", bufs=1))
            fcp = fctx.enter_context(tc.tile_pool(name="fcp", bufs=1))
            zp = fctx.enter_context(tc.tile_pool(name="zp", bufs=1))
            fwork = fctx.enter_context(tc.tile_pool(name="fwork", bufs=2))

            fwt = {}
            for name in ('FC1', 'FC2', 'FC2B', 'FC3', 'FC3B', 'FC4', 'FC4B', 'FC5B'):
                ap = wd[name]
                shape = ap.shape
                if len(shape) == 2:
                    t = fwp.tile(list(shape), F32, tag=f"w_{name}", name=f"w_{name}")
                    nc.sync.dma_start(t[:], ap[:])
                    fwt[name] = t
                else:
                    tiles = []
                    for i in range(shape[0]):
                        t = fwp.tile(list(shape[1:]), F32, tag=f"w_{name}_{i}",
                                     name=f"w_{name}_{i}")
                        nc.sync.dma_start(t[:], ap[i])
                        tiles.append(t)
                    fwt[name] = tiles
            fc5a = fwp.tile([64, 3], F32, tag="fc5a", name="fc5a")
            fc5b = fwp.tile([64, 3], F32, tag="fc5b", name="fc5b")
            nc.sync.dma_start(fc5a[:], wd['FC5'][0:64, :])
            nc.sync.dma_start(fc5b[:], wd['FC5'][64:128, :])

            # FCin 15 tiles [128, BC]
            fcin = [fcp.tile([128, BC], F32, tag=f"fcin{t}", name=f"fcin{t}")
                    for t in range(15)]
            for t in range(15):
                nc.vector.memset(fcin[t][:], 0.0)
            fbv = fball.rearrange("p (b c) -> p b c", c=14)
            for ch in range(14):
                kt, kp = divmod(ch * 64, 128)
                nc.any.tensor_copy(fcin[kt][kp:kp + 24, :], fbv[0:24, :, ch])
                nc.any.tensor_copy(fcin[kt][kp + 32:kp + 56, :], fbv[32:56, :, ch])
                nc.any.tensor_copy(fcin[7 + kt][kp:kp + 32, :], fbv[64:96, :, ch])
                nc.any.tensor_copy(fcin[7 + kt][kp + 32:kp + 64, :], fbv[96:128, :, ch])
            nc.vector.memset(fcin[14][0:1, :], 1.0)

            def fc_matmul(wtiles, ktiles, mts, Mp, tag):
                outs = {}
                for mt in mts:
                    pt = ps.tile([Mp, BC], F32, tag="cps", name="cps", bufs=6)
                    for k, (wtile, ktile) in enumerate(zip(wtiles, ktiles)):
                        nc.tensor.matmul(pt[:], wtile[:, mt * Mp:(mt + 1) * Mp], ktile,
                                         start=(k == 0), stop=(k == len(ktiles) - 1))
                    outs[mt] = pt
                return outs

            # FC1
            z1ps = fc_matmul(fwt['FC1'], [t[:] for t in fcin], [0, 1], 128, "fc1")
            z1re = zp.tile([128, BC], F32, tag="z1re", name="z1re")
            z1im = zp.tile([128, BC], F32, tag="z1im", name="z1im")
            cleaky(fwork, z1ps[0][:], z1ps[1][:], 128, BC, out_re=z1re[:], out_im=z1im[:])
            # FC2: pairs (0,4),(1,5),(2,6),(3,7)
            z2 = []
            for i in range(4):
                zre = zp.tile([128, BC], F32, tag=f"z2re{i}", name=f"z2re{i}")
                zim = zp.tile([128, BC], F32, tag=f"z2im{i}", name=f"z2im{i}")
                pp = fc_matmul([fwt['FC2'][0], fwt['FC2'][1], fwt['FC2B']],
                               [z1re[:], z1im[:], ones[:]], [i, 4 + i], 128, "fc2")
                cleaky(fwork, pp[i][:], pp[4 + i][:], 128, BC, out_re=zre[:], out_im=zim[:])
                z2.append((zre, zim))
            # FC3
            z3ps = fc_matmul(fwt['FC3'] + [fwt['FC3B']],
                             [z2[i][0][:] for i in range(4)] + [z2[i][1][:] for i in range(4)] + [ones[:]],
                             [0, 1], 128, "fc3")
            z3re = zp.tile([128, BC], F32, tag="z3re", name="z3re")
            z3im = zp.tile([128, BC], F32, tag="z3im", name="z3im")
            cleaky(fwork, z3ps[0][:], z3ps[1][:], 128, BC, out_re=z3re[:], out_im=z3im[:])
            # FC4
            z4ps = fc_matmul([fwt['FC4'][0], fwt['FC4'][1], fwt['FC4B']],
                             [z3re[:], z3im[:], ones[:]], [0, 1], 64, "fc4")
            z4re = zp.tile([64, BC], F32, tag="z4re", name="z4re")
            z4im = zp.tile([64, BC], F32, tag="z4im", name="z4im")
            cleaky(fwork, z4ps[0][:], z4ps[1][:], 64, BC, out_re=z4re[:], out_im=z4im[:])
            # FC5 (re only, M=3)
            p5 = ps.tile([3, BC], F32, tag="cps", name="cps", bufs=6)
            nc.tensor.matmul(p5[:], fc5a[:], z4re[:], start=True, stop=False)
            nc.tensor.matmul(p5[:], fc5b[:], z4im[:], start=False, stop=False)
            nc.tensor.matmul(p5[:], fwt['FC5B'][:], ones[:], start=False, stop=True)
            osb = zp.tile([3, BC], F32, tag="osb", name="osb")
            nc.scalar.activation(osb[:], p5[:], AF.Sigmoid)
            nc.sync.dma_start(out_d.rearrange("b h -> h b"), osb[:])


# ---------------------------------------------------------------- entry point
def kernel(**inputs):
    if 'nc' not in _CACHE:
        _CACHE['nc'] = _build_program()
    nc = _CACHE['nc']
    w = _build_weights(inputs)
    x = np.ascontiguousarray(inputs['x'], dtype=np.float32)
    in_maps = []
    for i in range(NCORES):
        m = {'x': x[i * BC:(i + 1) * BC]}
        m.update(w)
        in_maps.append(m)
    res = run_bass_kernel_spmd(nc, in_maps, list(range(NCORES)))
    out = np.concatenate([res.results[i]['out'] for i in range(NCORES)], axis=0)
    return out.astype(np.float32)


if __name__ == "__main__":
    d = np.load('/root/problem/ref_inputs.npz')
    inp = {k: d[k] for k in d.files}
    expected = np.load('/root/problem/ref_expected.npy')
    got = kernel(**inp)
    err = np.abs(got - expected)
    print("max abs err:", err.max(), " rel:", err.max() / np.abs(expected).max())


# revision 10
# speedup vs baseline: 51.1885x; 51.1885x over previous
"""Trainium2 Bass kernel for nn_CONV_COMPLEX_v1 (complex CNN + FC heads).

Self-contained: builds all Toeplitz/block-diag weights in numpy, compiles one
SPMD Bass/Tile program, shards batch over 8 cores, gathers [4096, 3] output.

See inline layout comments; validated against the reference via layouts.py
(numpy mirror) to ~1e-7.
"""
import sys
if '/opt/trn_rl_repo' not in sys.path:
    sys.path.insert(0, '/opt/trn_rl_repo')

import numpy as np
import concourse.bass as bass
import concourse.bacc as bacc
import concourse.mybir as mybir
from concourse import tile, dve_ops
from concourse.dve_spec import Spec, Src0, Src1, C0, C1, select
from concourse.dve_ops import RECIPROCAL_APPROX_FAST
from concourse.bass_utils import run_bass_kernel_spmd

AF = mybir.ActivationFunctionType
OP = mybir.AluOpType
dt = mybir.dt
F32 = dt.float32

SLOPE = 0.05
B, T, C = 4096, 64, 14
NCORES = 8
BC = B // NCORES          # 512 batch per core
R = BC * 14               # 7168 rows per core
CH = 512                  # row chunk
NCHUNK = R // CH          # 14
BIG = 1e30

# ---------------------------------------------------------------- custom op
def _register(op):
    if op.name in dve_ops._SUB_OPCODE_FOR_NAME:
        return
    dve_ops.OPS.append(op)
    dve_ops._SUB_OPCODE_FOR_NAME[op.name] = max(dve_ops._SUB_OPCODE_FOR_NAME.values()) + 1
    dve_ops.CUSTOM_DVE_SPECS[op.name] = op.spec


CLEAKY_SELMUL = dve_ops.DveOp(
    "CLEAKY_SELMUL",
    Spec(body=select(Src1 < C0, Src0 * Src1, C1),
         reference=lambda in0, in1, s0, s1, imm2: np.where(
             in1 < s0, (in0 * in1).astype(np.float32), np.float32(s1)).astype(np.float32)),
    subdim=False,
    uops_sha={"v3": "7defef3488ac79a9", "v4": "1911aa2b7a375206"},
)
_register(CLEAKY_SELMUL)

# ---------------------------------------------------------------- weights (numpy)
GROUPS = [
    (0, 3), (1, 4), (2, 5), (None, 6),
    (0, 7), (1, 8), (2, 9), (None, 10),
    ((0, 1), 13), ((1, 2), 13), ((2, 0), 13),
    ((0, 1), 11), ((1, 2), 12), ((2, 0), 13),
]


def _xn_row(comp, t):
    if t < 32:
        return comp * 32 + t
    return 64 + comp * 32 + (t - 32)


def _build_weights(inp):
    w = {}
    # conv32 (branch1): W32 [129, 64]: cols 0:32 re-out (co*3+p, pad 24:32), 32:64 im-out
    wr, wi = inp['conv32_w'].real, inp['conv32_w'].imag
    br, bi = inp['conv32_b'].real, inp['conv32_b'].imag
    W32 = np.zeros((129, 64), np.float32)
    for co in range(8):
        for p in range(3):
            m = co * 3 + p
            for k in range(32):
                t = 16 * p + k
                W32[_xn_row(0, t), m] += wr[co, 0, k]
                W32[_xn_row(1, t), m] += -wi[co, 0, k]
                W32[_xn_row(0, t), 32 + m] += wi[co, 0, k]
                W32[_xn_row(1, t), 32 + m] += wr[co, 0, k]
            W32[128, m] = br[co]
            W32[128, 32 + m] = bi[co]
    W32[128, 56:64] = 1.0
    w['W32'] = W32

    # conv1: W1 [65, 1024], tile q cols q*128..; q = parity*4 + comp_o*2 + cihalf
    wr, wi = inp['c3w1'].real, inp['c3w1'].imag
    br, bi = inp['c3b1'].real, inp['c3b1'].imag
    W1 = np.zeros((65, 1024), np.float32)
    for parity in range(2):
        for comp_o in range(2):
            for cihalf in range(2):
                q = parity * 4 + comp_o * 2 + cihalf
                for ci_loc in range(8):
                    ci = cihalf * 8 + ci_loc
                    for l2 in range(16):
                        l = 2 * l2 + parity
                        m = q * 128 + ci_loc * 16 + l2
                        for k in range(3):
                            tp = (l + k - 1) % 32
                            if comp_o == 0:
                                W1[tp, m] += wr[ci, 0, k]
                                W1[32 + tp, m] += -wi[ci, 0, k]
                            else:
                                W1[tp, m] += wi[ci, 0, k]
                                W1[32 + tp, m] += wr[ci, 0, k]
                        W1[64, m] = br[ci] if comp_o == 0 else bi[ci]
    w['W1'] = W1

    # conv2: W2 [4, 128, 1024] kc-major; W2B [1, 1024]
    wr, wi = inp['c3w2'].real, inp['c3w2'].imag
    br, bi = inp['c3b2'].real, inp['c3b2'].imag
    W2 = np.zeros((4, 128, 1024), np.float32)
    W2B = np.zeros((1, 1024), np.float32)
    for comp_o in range(2):
        for parity in range(2):
            for l3half in range(2):
                mt = comp_o * 4 + parity * 2 + l3half
                for l3_loc in range(4):
                    l = 2 * (l3half * 4 + l3_loc) + parity
                    for co in range(32):
                        m = mt * 128 + l3_loc * 32 + co
                        for ci in range(16):
                            for k in range(3):
                                li = (l + k - 1) % 16
                                krow = (ci % 8) * 16 + li
                                if comp_o == 0:
                                    W2[0 + ci // 8, krow, m] += wr[co, ci, k]
                                    W2[2 + ci // 8, krow, m] += -wi[co, ci, k]
                                else:
                                    W2[0 + ci // 8, krow, m] += wi[co, ci, k]
                                    W2[2 + ci // 8, krow, m] += wr[co, ci, k]
                        W2B[0, m] = br[co] if comp_o == 0 else bi[co]
    w['W2'] = W2
    w['W2B'] = W2B

    # conv3: W3 [4, 128, 512]; W3B [1, 512]; M tiles: mt = comp_o*2 + lhalf, part = l_loc*32+co
    wr, wi = inp['c3w3'].real, inp['c3w3'].imag
    br, bi = inp['c3b3'].real, inp['c3b3'].imag
    W3 = np.zeros((4, 128, 512), np.float32)
    W3B = np.zeros((1, 512), np.float32)
    for comp_o in range(2):
        for lhalf in range(2):
            mt = comp_o * 2 + lhalf
            for l_loc in range(4):
                l = lhalf * 4 + l_loc
                for co in range(32):
                    m = mt * 128 + l_loc * 32 + co
                    for ci in range(32):
                        for k in range(3):
                            j = (l + k - 1) % 8
                            kc = 0 if j < 4 else 1
                            krow = (j % 4) * 32 + ci
                            if comp_o == 0:
                                W3[0 + kc, krow, m] += wr[co, ci, k]
                                W3[2 + kc, krow, m] += -wi[co, ci, k]
                            else:
                                W3[0 + kc, krow, m] += wi[co, ci, k]
                                W3[2 + kc, krow, m] += wr[co, ci, k]
                    W3B[0, m] = br[co] if comp_o == 0 else bi[co]
    w['W3'] = W3
    w['W3B'] = W3B

    # FC1: [15, 128, 256]; K idx = ch*64 + comp*32 + f (blocks of 7 tiles each), tile14 = bias
    wr, wi = inp['hw1'].real, inp['hw1'].imag
    br, bi = inp['hb1'].real, inp['hb1'].imag
    FC1 = np.zeros((15, 128, 256), np.float32)
    for h in range(3):
        for o in range(40):
            m = h * 40 + o
            for f in range(56):
                for ch in range(14):
                    i = f * 14 + ch
                    if f < 24:
                        base, fl = 0, f
                    else:
                        base, fl = 7, f - 24
                    kr = ch * 64 + 0 * 32 + fl
                    ki = ch * 64 + 1 * 32 + fl
                    FC1[base + kr // 128, kr % 128, m] += wr[h, o, i]
                    FC1[base + ki // 128, ki % 128, m] += -wi[h, o, i]
                    FC1[base + kr // 128, kr % 128, 128 + m] += wi[h, o, i]
                    FC1[base + ki // 128, ki % 128, 128 + m] += wr[h, o, i]
            FC1[14, 0, m] = br[h, o]
            FC1[14, 0, 128 + m] = bi[h, o]
    FC1[14, 0, 248:256] = 1.0
    w['FC1'] = FC1

    # FC2: [2, 128, 1024] (k-tile0 from z1re, 1 from z1im); FC2B [1, 1024]
    wr, wi = inp['hw2'].real, inp['hw2'].imag
    br, bi = inp['hb2'].real, inp['hb2'].imag
    FC2 = np.zeros((2, 128, 1024), np.float32)
    FC2B = np.zeros((1, 1024), np.float32)
    for h in range(3):
        for o in range(160):
            m = h * 160 + o
            for i in range(40):
                k = h * 40 + i
                FC2[0, k, m] += wr[h, o, i]
                FC2[1, k, m] += -wi[h, o, i]
                FC2[0, k, 512 + m] += wi[h, o, i]
                FC2[1, k, 512 + m] += wr[h, o, i]
            FC2B[0, m] = br[h, o]
            FC2B[0, 512 + m] = bi[h, o]
    FC2B[0, 992:1024] = 1.0
    w['FC2'] = FC2
    w['FC2B'] = FC2B

    # FC3: [8, 128, 256] (tiles 0..3 = z2re rows, 4..7 = z2im); FC3B [1, 256]
    wr, wi = inp['hw3'].real, inp['hw3'].imag
    br, bi = inp['hb3'].real, inp['hb3'].imag
    FC3 = np.zeros((8, 128, 256), np.float32)
    FC3B = np.zeros((1, 256), np.float32)
    for h in range(3):
        for o in range(40):
            m = h * 40 + o
            for i in range(160):
                k = h * 160 + i
                FC3[k // 128, k % 128, m] += wr[h, o, i]
                FC3[4 + k // 128, k % 128, m] += -wi[h, o, i]
                FC3[k // 128, k % 128, 128 + m] += wi[h, o, i]
                FC3[4 + k // 128, k % 128, 128 + m] += wr[h, o, i]
            FC3B[0, m] = br[h, o]
            FC3B[0, 128 + m] = bi[h, o]
    FC3B[0, 248:256] = 1.0
    w['FC3'] = FC3
    w['FC3B'] = FC3B

    # FC4: [2, 128, 128] (tile0 z3re, tile1 z3im); FC4B [1, 128]; M: re 0:64 (60+4pad), im 64:128
    wr, wi = inp['hw4'].real, inp['hw4'].imag
    br, bi = inp['hb4'].real, inp['hb4'].imag
    FC4 = np.zeros((2, 128, 128), np.float32)
    FC4B = np.zeros((1, 128), np.float32)
    for h in range(3):
        for o in range(20):
            m = h * 20 + o
            for i in range(40):
                k = h * 40 + i
                FC4[0, k, m] += wr[h, o, i]
                FC4[1, k, m] += -wi[h, o, i]
                FC4[0, k, 64 + m] += wi[h, o, i]
                FC4[1, k, 64 + m] += wr[h, o, i]
            FC4B[0, m] = br[h, o]
            FC4B[0, 64 + m] = bi[h, o]
    FC4B[0, 124:128] = 1.0
    w['FC4'] = FC4
    w['FC4B'] = FC4B

    # FC5: [128, 3] (rows 0:64 z4re, 64:128 z4im); FC5B [1, 3]
    wr, wi = inp['hw5'].real, inp['hw5'].imag
    br = inp['hb5'].real
    FC5 = np.zeros((128, 3), np.float32)
    FC5B = np.zeros((1, 3), np.float32)
    for h in range(3):
        for i in range(20):
            k = h * 20 + i
            FC5[k, h] += wr[h, 0, i]
            FC5[64 + k, h] += -wi[h, 0, i]
        FC5B[0, h] = br[h, 0]
    w['FC5'] = FC5
    w['FC5B'] = FC5B
    return w


# ---------------------------------------------------------------- bass program
_CACHE = {}


def _build_program(repeat=1):
    nc = bacc.Bacc("TRN2", target_bir_lowering=False, debug=False, num_devices=NCORES)

    x_d = nc.dram_tensor("x", [BC, T, C], F32, kind="ExternalInput").ap()
    wd = {}
    for name, shape in [
        ('W32', [129, 64]), ('W1', [65, 1024]),
        ('W2', [4, 128, 1024]), ('W2B', [1, 1024]),
        ('W3', [4, 128, 512]), ('W3B', [1, 512]),
        ('FC1', [15, 128, 256]), ('FC2', [2, 128, 1024]), ('FC2B', [1, 1024]),
        ('FC3', [8, 128, 256]), ('FC3B', [1, 256]),
        ('FC4', [2, 128, 128]), ('FC4B', [1, 128]),
        ('FC5', [128, 3]), ('FC5B', [1, 3]),
    ]:
        wd[name] = nc.dram_tensor(name, shape, F32, kind="ExternalInput").ap()
    out_d = nc.dram_tensor("out", [BC, 3], F32, kind="ExternalOutput").ap()

    with tile.TileContext(nc) as tc:
        for _ in range(repeat):
            _emit(nc, tc, x_d, wd, out_d)

    nc.compile()
    return nc


def _emit(nc, tc, x_d, wd, out_d):
    import contextlib

    # ---------------- global pools
    with contextlib.ExitStack() as gctx:
        cpool = gctx.enter_context(tc.tile_pool(name="consts", bufs=1))
        fb = gctx.enter_context(tc.tile_pool(name="fb", bufs=1))
        ps = gctx.enter_context(tc.tile_pool(name="ps", bufs=1, space="PSUM"))

        ones = cpool.tile([1, CH], F32, tag="ones", name="ones")
        nc.gpsimd.memset(ones[:], 1.0)
        bias_s = cpool.tile([128, 1], F32, tag="bias_s", name="bias_s")
        bias_c = cpool.tile([128, 1], F32, tag="bias_c", name="bias_c")
        nc.gpsimd.memset(bias_s[:], 0.475 * np.pi)
        nc.gpsimd.memset(bias_c[:], 0.025 * np.pi)

        # Fball [128, R]: 0:24 b1re, 32:56 b1im, 64:96 b2re, 96:128 b2im
        fball = fb.tile([128, R], F32, tag="fball", name="fball")

        def cleaky(pool, re_ap, im_ap, P, n, out_re=None, out_im=None, bufs=2):
            rcp = pool.tile([P, n], F32, tag="ck_rcp", name="ck_rcp", bufs=bufs)
            nc.vector.reciprocal_approx_fast(out=rcp[:], in_=im_ap)
            g = pool.tile([P, n], F32, tag="ck_g", name="ck_g", bufs=bufs)
            nc.vector._custom_dve(CLEAKY_SELMUL, out=g[:], in0=re_ap, in1=rcp[:],
                                  s0=0.0, s1=-BIG)
            b = pool.tile([P, n], F32, tag="ck_b", name="ck_b", bufs=bufs)
            nc.scalar.activation(b[:], g[:], AF.Arctan)
            s = pool.tile([P, n], F32, tag="ck_s", name="ck_s", bufs=bufs)
            c = pool.tile([P, n], F32, tag="ck_c", name="ck_c", bufs=bufs)
            nc.scalar.activation(s[:], b[:], AF.Sin, scale=0.95, bias=bias_s[0:P, :])
            nc.scalar.activation(c[:], b[:], AF.Sin, scale=-0.95, bias=bias_c[0:P, :])
            p1 = pool.tile([P, n], F32, tag="ck_p1", name="ck_p1", bufs=bufs)
            q1 = pool.tile([P, n], F32, tag="ck_q1", name="ck_q1", bufs=bufs)
            if out_re is None:
                ore_t = pool.tile([P, n], F32, tag="ck_ore", name="ck_ore", bufs=bufs)
                out_re = ore_t[:]
            else:
                ore_t = None
            if out_im is None:
                oim_t = pool.tile([P, n], F32, tag="ck_oim", name="ck_oim", bufs=bufs)
                out_im = oim_t[:]
            else:
                oim_t = None
            nc.vector.tensor_tensor(out=p1[:], in0=re_ap, in1=c[:], op=OP.mult)
            nc.vector.tensor_tensor(out=q1[:], in0=im_ap, in1=s[:], op=OP.mult)
            nc.vector.tensor_tensor(out=out_re, in0=p1[:], in1=q1[:], op=OP.subtract)
            p2 = pool.tile([P, n], F32, tag="ck_p1", name="ck_p1", bufs=bufs)
            q2 = pool.tile([P, n], F32, tag="ck_q1", name="ck_q1", bufs=bufs)
            nc.vector.tensor_tensor(out=p2[:], in0=re_ap, in1=s[:], op=OP.mult)
            nc.vector.tensor_tensor(out=q2[:], in0=im_ap, in1=c[:], op=OP.add if False else OP.mult)
            nc.vector.tensor_tensor(out=out_im, in0=p2[:], in1=q2[:], op=OP.add)
            return out_re, out_im

        # ================= conv phase =================
        with contextlib.ExitStack() as cctx:
            big = cctx.enter_context(tc.tile_pool(name="big", bufs=1))

            # XN build (two 64-partition tiles: lo = t 0..31, hi = t 32..63)
            xnlo = big.tile([64, R], F32, tag="xnlo", name="xnlo")
            xnhi = big.tile([64, R], F32, tag="xnhi", name="xnhi")
            with tc.tile_pool(name="x0pool", bufs=1) as x0pool:
                x0 = x0pool.tile([64, R], F32, tag="x0", name="x0")
                nc.sync.dma_start(x0.rearrange("t (b c) -> t b c", c=14),
                                  x_d.rearrange("b t c -> t b c"))

                def xplane(c, lo):
                    return x0[lo:lo + 32].rearrange("t (b c) -> t b c", c=14)[:, :, c]

                for ch, (respec, imc) in enumerate(GROUPS):
                    for lo, xt_ in ((0, xnlo), (32, xnhi)):
                        dst = xt_.rearrange("p (b c) -> p b c", c=14)
                        if respec is None:
                            nc.vector.memset(dst[0:32, :, ch], 0.0)
                        elif isinstance(respec, tuple):
                            nc.any.tensor_tensor(out=dst[0:32, :, ch],
                                                 in0=xplane(respec[0], lo),
                                                 in1=xplane(respec[1], lo), op=OP.subtract)
                        else:
                            nc.any.tensor_copy(dst[0:32, :, ch], xplane(respec, lo))
                        nc.any.tensor_copy(dst[32:64, :, ch], xplane(imc, lo))

            wpool = cctx.enter_context(tc.tile_pool(name="cw", bufs=1))
            work = cctx.enter_context(tc.tile_pool(name="cwork", bufs=2))
            x2p = cctx.enter_context(tc.tile_pool(name="x2p", bufs=1))

            # conv-phase weights
            wt = {}
            for name in ('W2', 'W2B', 'W3', 'W3B'):
                ap = wd[name]
                shape = ap.shape
                if len(shape) == 2:
                    t = wpool.tile(list(shape), F32, tag=f"w_{name}", name=f"w_{name}")
                    nc.sync.dma_start(t[:], ap[:])
                    wt[name] = t
                else:
                    tiles = []
                    for i in range(shape[0]):
                        t = wpool.tile(list(shape[1:]), F32, tag=f"w_{name}_{i}",
                                       name=f"w_{name}_{i}")
                        nc.sync.dma_start(t[:], ap[i])
                        tiles.append(t)
                    wt[name] = tiles
            w32lo = wpool.tile([64, 64], F32, tag="w32lo", name="w32lo")
            w32hi = wpool.tile([64, 64], F32, tag="w32hi", name="w32hi")
            w32b = wpool.tile([1, 64], F32, tag="w32b", name="w32b")
            nc.sync.dma_start(w32lo[:], wd['W32'][0:64, :])
            nc.sync.dma_start(w32hi[:], wd['W32'][64:128, :])
            nc.sync.dma_start(w32b[:], wd['W32'][128:129, :])
            w1a = wpool.tile([64, 1024], F32, tag="w1a", name="w1a")
            w1b = wpool.tile([1, 1024], F32, tag="w1b", name="w1b")
            nc.sync.dma_start(w1a[:], wd['W1'][0:64, :])
            nc.sync.dma_start(w1b[:], wd['W1'][64:65, :])

            for cidx in range(NCHUNK):
                cs = slice(cidx * CH, (cidx + 1) * CH)
                rhs_lo = xnlo[:, cs]
                rhs_hi = xnhi[:, cs]

                # --- branch1 conv32
                p_re = ps.tile([32, CH], F32, tag="cps", name="cps", bufs=6)
                p_im = ps.tile([32, CH], F32, tag="cps", name="cps", bufs=6)
                nc.tensor.matmul(p_re[:], w32lo[:, 0:32], rhs_lo, start=True, stop=False)
                nc.tensor.matmul(p_re[:], w32hi[:, 0:32], rhs_hi, start=False, stop=False)
                nc.tensor.matmul(p_re[:], w32b[:, 0:32], ones[:], start=False, stop=True)
                nc.tensor.matmul(p_im[:], w32lo[:, 32:64], rhs_lo, start=True, stop=False)
                nc.tensor.matmul(p_im[:], w32hi[:, 32:64], rhs_hi, start=False, stop=False)
                nc.tensor.matmul(p_im[:], w32b[:, 32:64], ones[:], start=False, stop=True)
                b1ore, b1oim = cleaky(work, p_re[:], p_im[:], 32, CH)
                nc.any.tensor_copy(fball[0:24, cs], b1ore[0:24, :])
                nc.any.tensor_copy(fball[32:56, cs], b1oim[0:24, :])

                # --- conv1: emit in cleaky-pair order (0,2),(1,3),(4,6),(5,7)
                c1 = [None] * 8
                X2 = [None] * 4
                for t in range(4):
                    X2[t] = x2p.tile([128, CH], F32, tag=f"x2_{t}", name=f"x2_{t}")

                def c1mm(q):
                    pt = ps.tile([128, CH], F32, tag="cps", name="cps", bufs=6)
                    nc.tensor.matmul(pt[:], w1a[:, q * 128:(q + 1) * 128], rhs_hi,
                                     start=True, stop=False)
                    nc.tensor.matmul(pt[:], w1b[:, q * 128:(q + 1) * 128], ones[:],
                                     start=False, stop=True)
                    c1[q] = pt

                ev = {}
                od = {}
                for cihalf in range(2):
                    c1mm(0 + cihalf); c1mm(2 + cihalf)
                    ev[cihalf] = cleaky(work, c1[cihalf][:], c1[2 + cihalf][:], 128, CH)
                    c1mm(4 + cihalf); c1mm(6 + cihalf)
                    od[cihalf] = cleaky(work, c1[4 + cihalf][:], c1[6 + cihalf][:], 128, CH)
                    nc.vector.tensor_tensor(out=X2[cihalf][:], in0=ev[cihalf][0],
                                            in1=od[cihalf][0], op=OP.max)
                    nc.vector.tensor_tensor(out=X2[2 + cihalf][:], in0=ev[cihalf][1],
                                            in1=od[cihalf][1], op=OP.max)

                # --- conv2: pairs (0,4),(2,6),(1,5),(3,7)
                c2 = [None] * 8
                X3 = [None] * 4
                for t in range(4):
                    X3[t] = x2p.tile([128, CH], F32, tag=f"x3_{t}", name=f"x3_{t}")

                def c2mm(mt):
                    pt = ps.tile([128, CH], F32, tag="cps", name="cps", bufs=6)
                    for kc in range(4):
                        nc.tensor.matmul(pt[:], wt['W2'][kc][:, mt * 128:(mt + 1) * 128],
                                         X2[kc][:], start=(kc == 0), stop=False)
                    nc.tensor.matmul(pt[:], wt['W2B'][:, mt * 128:(mt + 1) * 128],
                                     ones[:], start=False, stop=True)
                    c2[mt] = pt

                pooled2 = {}
                for l3half in range(2):
                    c2mm(0 + l3half); c2mm(4 + l3half)
                    e_ = cleaky(work, c2[0 + l3half][:], c2[4 + l3half][:], 128, CH)
                    c2mm(2 + l3half); c2mm(6 + l3half)
                    o_ = cleaky(work, c2[2 + l3half][:], c2[6 + l3half][:], 128, CH)
                    nc.vector.tensor_tensor(out=X3[l3half][:], in0=e_[0], in1=o_[0], op=OP.max)
                    nc.vector.tensor_tensor(out=X3[2 + l3half][:], in0=e_[1], in1=o_[1], op=OP.max)

                # --- conv3: pairs (0,2),(1,3)
                c3 = [None] * 4

                def c3mm(mt):
                    pt = ps.tile([128, CH], F32, tag="cps", name="cps", bufs=6)
                    for kc in range(4):
                        nc.tensor.matmul(pt[:], wt['W3'][kc][:, mt * 128:(mt + 1) * 128],
                                         X3[kc][:], start=(kc == 0), stop=False)
                    nc.tensor.matmul(pt[:], wt['W3B'][:, mt * 128:(mt + 1) * 128],
                                     ones[:], start=False, stop=True)
                    c3[mt] = pt

                c3mm(0); c3mm(2)
                R0c, I0c = cleaky(work, c3[0][:], c3[2][:], 128, CH)
                c3mm(1); c3mm(3)
                R1c, I1c = cleaky(work, c3[1][:], c3[3][:], 128, CH)
                for src0, src1, fbase in ((R0c, R1c, 64), (I0c, I1c, 96)):
                    G = work.tile([128, CH], F32, tag="c3g", name="c3g", bufs=1)
                    nc.vector.tensor_tensor(out=G[:], in0=src0, in1=src1, op=OP.max)
                    t64 = work.tile([64, CH], F32, tag="c3t64", name="c3t64", bufs=1)
                    nc.any.tensor_copy(t64[:], G[64:128, :])
                    H = work.tile([64, CH], F32, tag="c3h", name="c3h", bufs=1)
                    nc.vector.tensor_tensor(out=H[:], in0=G[0:64, :], in1=t64[:], op=OP.max)
                    t32 = work.tile([32, CH], F32, tag="c3t32", name="c3t32", bufs=1)
                    nc.any.tensor_copy(t32[:], H[32:64, :])
                    Fx = work.tile([32, CH], F32, tag="c3f", name="c3f", bufs=1)
                    nc.vector.tensor_tensor(out=Fx[:], in0=H[0:32, :], in1=t32[:], op=OP.max)
                    nc.any.tensor_copy(fball[fbase:fbase + 32, cs], Fx[:])

        # ================= FC phase =================
        with contextlib.ExitStack() as fctx:
            fwp = fctx.enter_context(tc.tile_pool(name="fw", bufs=1))
            fcp = fctx.enter_context(tc.tile_pool(name="fcp", bufs=1))
            zp = fctx.enter_context(tc.tile_pool(name="zp", bufs=1))
            fwork = fctx.enter_context(tc.tile_pool(name="fwork", bufs=2))

            fwt = {}
            for name in ('FC1', 'FC2', 'FC2B', 'FC3', 'FC3B', 'FC4', 'FC4B', 'FC5B'):
                ap = wd[name]
                shape = ap.shape
                if len(shape) == 2:
                    t = fwp.tile(list(shape), F32, tag=f"w_{name}", name=f"w_{name}")
                    nc.sync.dma_start(t[:], ap[:])
                    fwt[name] = t
                else:
                    tiles = []
                    for i in range(shape[0]):
                        t = fwp.tile(list(shape[1:]), F32, tag=f"w_{name}_{i}",
                                     name=f"w_{name}_{i}")
                        nc.sync.dma_start(t[:], ap[i])
                        tiles.append(t)
                    fwt[name] = tiles
            fc5a = fwp.tile([64, 3], F32, tag="fc5a", name="fc5a")
            fc5b = fwp.tile([64, 3], F32, tag="fc5b", name="fc5b")
            nc.sync.dma_start(fc5a[:], wd['FC5'][0:64, :])
            nc.sync.dma_start(fc5b[:], wd['FC5'][64:128, :])

            # FCin 15 tiles [128, BC]
            fcin = [fcp.tile([128, BC], F32, tag=f"fcin{t}", name=f"fcin{t}")
                    for t in range(15)]
            for t in range(15):
                nc.vector.memset(fcin[t][:], 0.0)
            fbv = fball.rearrange("p (b c) -> p b c", c=14)
            for ch in range(14):
                kt, kp = divmod(ch * 64, 128)
                nc.any.tensor_copy(fcin[kt][kp:kp + 24, :], fbv[0:24, :, ch])
                nc.any.tensor_copy(fcin[kt][kp + 32:kp + 56, :], fbv[32:56, :, ch])
                nc.any.tensor_copy(fcin[7 + kt][kp:kp + 32, :], fbv[64:96, :, ch])
                nc.any.tensor_copy(fcin[7 + kt][kp + 32:kp + 64, :], fbv[96:128, :, ch])
            nc.vector.memset(fcin[14][0:1, :], 1.0)

            def fc_matmul(wtiles, ktiles, mts, Mp, tag):
                outs = {}
                for mt in mts:
                    pt = ps.tile([Mp, BC], F32, tag="cps", name="cps", bufs=6)
                    for k, (wtile, ktile) in enumerate(zip(wtiles, ktiles)):
                        nc.tensor.matmul(pt[:], wtile[:, mt * Mp:(mt + 1) * Mp], ktile,
                                         start=(k == 0), stop=(k == len(ktiles) - 1))
                    outs[mt] = pt
                return outs

            # FC1
            z1ps = fc_matmul(fwt['FC1'], [t[:] for t in fcin], [0, 1], 128, "fc1")
            z1re = zp.tile([128, BC], F32, tag="z1re", name="z1re")
            z1im = zp.tile([128, BC], F32, tag="z1im", name="z1im")
            cleaky(fwork, z1ps[0][:], z1ps[1][:], 128, BC, out_re=z1re[:], out_im=z1im[:])
            # FC2: pairs (0,4),(1,5),(2,6),(3,7)
            z2 = []
            for i in range(4):
                zre = zp.tile([128, BC], F32, tag=f"z2re{i}", name=f"z2re{i}")
                zim = zp.tile([128, BC], F32, tag=f"z2im{i}", name=f"z2im{i}")
                pp = fc_matmul([fwt['FC2'][0], fwt['FC2'][1], fwt['FC2B']],
                               [z1re[:], z1im[:], ones[:]], [i, 4 + i], 128, "fc2")
                cleaky(fwork, pp[i][:], pp[4 + i][:], 128, BC, out_re=zre[:], out_im=zim[:])
                z2.append((zre, zim))
            # FC3
            z3ps = fc_matmul(fwt['FC3'] + [fwt['FC3B']],
                             [z2[i][0][:] for i in range(4)] + [z2[i][1][:] for i in range(4)] + [ones[:]],
                             [0, 1], 128, "fc3")
            z3re = zp.tile([128, BC], F32, tag="z3re", name="z3re")
            z3im = zp.tile([128, BC], F32, tag="z3im", name="z3im")
            cleaky(fwork, z3ps[0][:], z3ps[1][:], 128, BC, out_re=z3re[:], out_im=z3im[:])
            # FC4
            z4ps = fc_matmul([fwt['FC4'][0], fwt['FC4'][1], fwt['FC4B']],
                             [z3re[:], z3im[:], ones[:]], [0, 1], 64, "fc4")
            z4re = zp.tile([64, BC], F32, tag="z4re", name="z4re")
            z4im = zp.tile([64, BC], F32, tag="z4im", name="z4im")
            cleaky(fwork, z4ps[0][:], z4ps[1][:], 64, BC, out_re=z4re[:], out_im=z4im[:])
            # FC5 (re only, M=3)
            p5 = ps.tile([3, BC], F32, tag="cps", name="cps", bufs=6)
            nc.tensor.matmul(p5[:], fc5a[:], z4re[:], start=True, stop=False)
            nc.tensor.matmul(p5[:], fc5b[:], z4im[:], start=False, stop=False)
            nc.tensor.matmul(p5[:], fwt['FC5B'][:], ones[:], start=False, stop=True)
            osb = zp.tile([3, BC], F32, tag="osb", name="osb")
            nc.scalar.activation(osb[:], p5[:], AF.Sigmoid)
            nc.sync.dma_start(out_d.rearrange("b h -> h b"), osb[:])


# ---------------------------------------------------------------- entry point
def kernel(**inputs):
    if 'nc' not in _CACHE:
        _CACHE['nc'] = _build_program()
    nc = _CACHE['nc']
    w = _build_weights(inputs)
    x = np.ascontiguousarray(inputs['x'], dtype=np.float32)
    in_maps = []
    for i in range(NCORES):
        m = {'x': x[i * BC:(i + 1) * BC]}
        m.update(w)
        in_maps.append(m)
    res = run_bass_kernel_spmd(nc, in_maps, list(range(NCORES)))
    out = np.concatenate([res.results[i]['out'] for i in range(NCORES)], axis=0)
    return out.astype(np.float32)


if __name__ == "__main__":
    d = np.load('/root/problem/ref_inputs.npz')
    inp = {k: d[k] for k in d.files}
    expected = np.load('/root/problem/ref_expected.npy')
    got = kernel(**inp)
    err = np.abs(got - expected)
    print("max abs err:", err.max(), " rel:", err.max() / np.abs(expected).max())
